# revision 4
# baseline (speedup 1.0000x reference)
"""MoE gate kernel for Trainium2 (8 NeuronCores, SPMD).

Computes, for hidden_states [4, 4096, 4096] f32 and gate_weight [8, 4096] f32:
    logits = hidden @ gate_weight.T          # [tokens, 8]
    p      = softmax(logits)                 # [tokens, 8]
    topk_w, topk_i = top_k(p, 2); topk_w /= topk_w.sum(-1, keepdims=True)

Sharding: data-parallel over tokens (B*S = 16384 -> 2048 tokens/core), gate
weight replicated.

Per-core pipeline (all engines overlapped):
  * DMA: natural-layout loads [128 tokens x 2*4096] (16 KiB descriptors,
    full HBM rate) -- the strided transpose-load used previously is
    descriptor-rate-bound and ~3x slower.
  * PE transposes each 128x128 h-chunk into PSUM (fp32, exact).
  * ACT evacuates each PSUM batch as xr = f32r(4096 * x^T)  (f32r is the
    PE's fast reduced-precision fp32 mode, ~tf32).
  * DVE computes the rounding residual xe = fp16(4096 * x^T - xr) in one
    fused scalar_tensor_tensor op (exact by Sterbenz; the 2^12 scale keeps
    xe out of fp16-denormal range).
  * Gate matmul per 256-token pair, orientation B (weights stationary):
      lgT[0:8]   += f32r(W)^T_c       @ xr_c      (1 cycle/row)
      lgT[32:40] += f32r(W - Wr)^T_c  @ xr_c      (same instruction: the
                    W-rounding-error correction rides in extra stationary
                    columns, zero marginal cost)
    then the X-rounding-error correction in fp16 (fast weight load):
      logits^T = fold(lgT) transposed back to [128t, 8e] via PE, and
      misc     += fp16(xe)^T_c @ fp16(W)_c  accumulates onto it in PSUM.
    Total error ~2^-22 relative: bit-comparable to full fp32 (relmax
    3.4e-06 vs reference, zero top-2 index mismatches).
  * Top-2 + renorm on DVE: w1 = 1/(1+exp((m2-m1)/4096)), w2 = 1-w1 (the
    softmax denominator cancels; /4096 undoes the scale inside the ACT
    Exp's free scale argument).

Walrus's TPB instruction encodings carry a single sync-wait slot, so a
post-pass hoists surplus Tile-generated waits onto same-engine
EventSemaphore prefix instructions (semantics-preserving).
"""

import numpy as np

H = 4096            # hidden size
E = 8               # experts
P = 128             # SBUF partitions
C = H // P          # 32 h-chunks of 128
T_TILE = 128        # tokens per tile (PSUM partition dim)
N_CORES = 8
TOKENS_TOTAL = 4 * 4096
TOKENS_PER_CORE = TOKENS_TOTAL // N_CORES   # 2048
N_TILES = TOKENS_PER_CORE // T_TILE         # 16

BEST_CONFIG = {
    "batches": (8, 8, 8, 8),
    "xtp_bufs": 8,
    "dma_tiles": 2,
    "nat_bufs": 3,
    "copy_pattern": "A",
}


def _legalize_sync_waits(nc, mybir):
    """Split surplus sync waits onto EventSemaphore prefix instructions."""
    limit = 1
    n = 0
    for bb in nc.main_func.blocks:
        out, changed = [], False
        for ins in bb.instructions:
            si = ins.sync_info
            if si is not None and len(si.on_wait) > limit:
                waits = list(si.on_wait)
                for w in waits[:-limit]:
                    es = mybir.InstEventSemaphore(
                        name=f"ESleg-{n}", engine=ins.engine, ins=[], outs=[],
                        sync_info=mybir.SyncInfo(on_wait=[w], on_update=[]),
                    )
                    out.append(es)
                    n += 1
                ins.sync_info = mybir.SyncInfo(
                    on_wait=waits[-limit:], on_update=list(si.on_update)
                )
                changed = True
            out.append(ins)
        if changed:
            bb.instructions = out
    return n


def build_program(tokens_per_core: int = TOKENS_PER_CORE, reps: int = 1,
                  legalize: bool = True, dma_tiles: int = 2,
                  batches=(8, 8, 8, 8), nat_bufs: int = 3,
                  xtp_bufs: int = 8, tp_bufs: int = 2, lg_bufs: int = 2,
                  pipe_depth: int = 1, copy_pattern: str = "A"):
    import concourse.bass as bass
    import concourse.mybir as mybir
    from concourse.masks import make_identity
    from concourse.tile import TileContext

    n_tiles = tokens_per_core // T_TILE
    assert n_tiles % (2 * dma_tiles) == 0 or dma_tiles == 2
    S = 4096.0  # keeps the fp16 X-residual out of denormal range
    nc = bass.Bass("TRN2", debug=False)
    x = nc.declare_dram_parameter(
        "x", [tokens_per_core, H], mybir.dt.float32, isOutput=False
    )
    w = nc.declare_dram_parameter("w", [E, H], mybir.dt.float32, isOutput=False)
    wq = nc.declare_dram_parameter(
        "wq", [P, n_tiles, 2], mybir.dt.float32, isOutput=True
    )
    iq = nc.declare_dram_parameter(
        "iq", [P, n_tiles, 2], mybir.dt.uint32, isOutput=True
    )

    # (n, t, a, h): token = (n*dma_tiles + a)*128 + t; 16 KiB DRAM runs
    x_r = x[:].rearrange("(n a t) h -> n t a h", t=T_TILE, a=dma_tiles)

    with TileContext(nc) as tc:
        with (
            tc.tile_pool(name="natpool", bufs=nat_bufs) as natpool,
            tc.tile_pool(name="xtp", bufs=xtp_bufs) as xtp,
            tc.tile_pool(name="cpool", bufs=1) as cpool,
            tc.tile_pool(name="tpsum", bufs=tp_bufs, space="PSUM") as tpsum,
            tc.tile_pool(name="lgp", bufs=lg_bufs, space="PSUM") as lgp,
        ):
            ident = cpool.tile([P, P], mybir.dt.float32)
            make_identity(nc, ident[:])
            # w_nat[p, c, e] = W[e, c*128+p]; 4-byte-descriptor gather DMA,
            # one-time setup cost outside the rep loop
            # issue on the ACT HWDGE ring so the token-data loads (sync/SP
            # ring) start immediately
            w_nat = cpool.tile([P, C, E], mybir.dt.float32)
            for e in range(E):
                nc.scalar.dma_start(
                    w_nat[:, :, e], w[e].rearrange("(c p) -> p c", p=P)
                )
            # wcat[:, c, 0:8] = f32r(W^T_c); [:, c, 32:40] = f32r(W - Wr)
            # (residual rows at partition 32 so the DVE fold's PSUM read is
            # 32-aligned; cols 8:32 are zero)
            wcat = cpool.tile([P, C, 4 * E + E], mybir.dt.float32r)
            nc.vector.memset(wcat[:].bitcast(mybir.dt.float32), 0.0)
            nc.vector.tensor_copy(wcat[:, :, 0:E], w_nat[:])
            we32 = cpool.tile([P, C, E], mybir.dt.float32)
            nc.vector.tensor_sub(
                we32[:], w_nat[:], wcat[:, :, 0:E].bitcast(mybir.dt.float32)
            )
            nc.vector.tensor_copy(wcat[:, :, 4 * E : 5 * E], we32[:])
            wf16 = cpool.tile([P, C, E], mybir.dt.float16)
            nc.vector.tensor_copy(wf16[:], w_nat[:])

            sorted_w = cpool.tile([P, n_tiles, E], mybir.dt.float32)
            idx_w = cpool.tile([P, n_tiles, E], mybir.dt.uint32)

            for _rep in range(reps):
                _emit_body(
                    nc, mybir, (natpool, xtp, tpsum, lgp),
                    (wcat, wf16, ident), (sorted_w, idx_w),
                    x_r, wq, iq, n_tiles, dma_tiles, batches, pipe_depth,
                    copy_pattern, S,
                )
    if legalize:
        _legalize_sync_waits(nc, mybir)
    return nc


def _emit_body(nc, mybir, pools, consts, outs, x_r, wq, iq, n_tiles,
               dma_tiles, batches, pipe_depth, copy_pattern, S):
    natpool, xtp, tpsum, lgp = pools
    wcat, wf16, ident = consts
    sorted_w, idx_w = outs
    assert sum(batches) == C

    xn_cur = [None]
    pending = {}          # pair idx -> list of (xr tile, xe tile, base, nb)
    copy_ctr = [0]

    def emit_trans(i):
        if i % dma_tiles == 0:
            xn = natpool.tile([P, dma_tiles, H], mybir.dt.float32)
            nc.sync.dma_start(xn[:], x_r[i // dma_tiles])
            xn_cur[0] = xn
        xn = xn_cur[0]
        a = i % dma_tiles
        done = 0
        for nb in batches:
            pst = tpsum.tile([P, nb, P], mybir.dt.float32, tag="b")
            for j in range(nb):
                c = done + j
                nc.tensor.transpose(
                    pst[:, j], xn[:, a, c * P : (c + 1) * P], ident[:]
                )
            if i % 2 == 0:
                xT = xtp.tile([P, nb, 2, P], mybir.dt.float32r)
                xE = xtp.tile([P, nb, 2, P], mybir.dt.float16, tag="xe")
                pending.setdefault(i // 2, []).append((xT, xE, done, nb))
            else:
                xT, xE = next(
                    (t, e2) for t, e2, b, n in pending[i // 2] if b == done
                )
            dst = xT[:, :, i % 2, :]
            eng = copy_pattern[copy_ctr[0] % len(copy_pattern)]
            # pass1: xr = f32r(S * x^T)
            if eng == "D":
                nc.vector.tensor_scalar_mul(dst, pst[:], S)
            else:
                nc.scalar.activation(
                    dst, pst[:], mybir.ActivationFunctionType.Copy, scale=S,
                )
            # pass2 (DVE): xe = fp16(S * x^T - xr); S*pst is exact in fp32,
            # the subtract is exact by Sterbenz, then rounded to fp16
            nc.vector.scalar_tensor_tensor(
                xE[:, :, i % 2, :], pst[:], S, dst.bitcast(mybir.dt.float32),
                op0=mybir.AluOpType.mult, op1=mybir.AluOpType.subtract,
            )
            copy_ctr[0] += 1
            done += nb

    def emit_mms(i):
        plist = pending.pop(i)
        lgT = lgp.tile([4 * E + E, 2, P], mybir.dt.float32, tag="lgT")
        for xT, xE, base, nb in plist:
            for j in range(nb):
                c = base + j
                nc.tensor.matmul(
                    lgT[:], wcat[:, c, :], xT[:, j],
                    start=(c == 0), stop=(c == C - 1),
                )
        # fold Wr-rows + Wer-rows while evacuating to SBUF (DVE may read
        # only one PSUM operand: stage the Wer rows via ACT first)
        lgT_e = xtp.tile([E, 2, P], mybir.dt.float32, tag="lgw", bufs=2)
        nc.scalar.activation(
            lgT_e[:], lgT[4 * E : 5 * E], mybir.ActivationFunctionType.Copy
        )
        lgT_sb = xtp.tile([E, 2, P], mybir.dt.float32, tag="lgsb", bufs=2)
        nc.vector.tensor_add(lgT_sb[:], lgT[0:E], lgT_e[:])
        # transpose main logits into misc[:, a, 8:16] FIRST (transpose mode
        # does not PSUM-accumulate), then the fp16 X-residual correction
        # matmuls accumulate onto it with start=False
        misc = lgp.tile([P, 2, 2 * E], mybir.dt.float32, tag="lgps")
        for a in range(2):
            nc.tensor.matmul(
                misc[:, a, E : 2 * E], lgT_sb[:, a, :], ident[0:E, 0:E],
                is_transpose=True, start=(a == 0), stop=False,
                skip_group_check=True,
            )
        for a in range(2):
            for xT, xE, base, nb in plist:
                for j in range(nb):
                    c = base + j
                    nc.tensor.matmul(
                        misc[:, a, E : 2 * E], xE[:, j, a, :], wf16[:, c, :],
                        start=False, stop=(a == 1 and c == C - 1),
                        skip_group_check=True,
                    )
        for a in range(2):
            t = 2 * i + a
            nc.vector.max(out=sorted_w[:, t], in_=misc[:, a, E : 2 * E])
            nc.vector.max_index(
                out=idx_w[:, t], in_max=sorted_w[:, t],
                in_values=misc[:, a, E : 2 * E],
            )

    n_pairs = n_tiles // 2
    for p in range(n_pairs):
        if p >= pipe_depth:
            emit_mms(p - pipe_depth)
        emit_trans(2 * p)
        emit_trans(2 * p + 1)
    for p in range(n_pairs - pipe_depth, n_pairs):
        emit_mms(p)

    # batched renorm on the 4096-scaled maxima: w1 = 1/(1+e^((m2-m1)/S)),
    # w2 = 1-w1 (softmax denominator cancels in the reference's top-k renorm)
    m1 = sorted_w[:, :, 0]
    m2 = sorted_w[:, :, 1]
    d = natpool.tile([P, n_tiles], mybir.dt.float32, tag="rn_d", bufs=1)
    nc.vector.tensor_sub(d[:], m2, m1)
    t = natpool.tile([P, n_tiles], mybir.dt.float32, tag="rn_t", bufs=1)
    nc.scalar.activation(
        t[:], d[:], mybir.ActivationFunctionType.Exp, scale=1.0 / S
    )
    denom = natpool.tile([P, n_tiles], mybir.dt.float32, tag="rn_dn", bufs=1)
    nc.vector.tensor_scalar_add(denom[:], t[:], 1.0)
    r = natpool.tile([P, n_tiles], mybir.dt.float32, tag="rn_r", bufs=1)
    nc.vector.reciprocal(r[:], denom[:])

    wout = natpool.tile([P, n_tiles, 2], mybir.dt.float32, tag="rn_w", bufs=1)
    nc.vector.tensor_copy(wout[:, :, 0], r[:])
    nc.vector.tensor_mul(wout[:, :, 1], t[:], r[:])
    iout = natpool.tile([P, n_tiles, 2], mybir.dt.uint32, tag="rn_i", bufs=1)
    nc.vector.tensor_copy(iout[:], idx_w[:, :, 0:2])

    nc.sync.dma_start(wq[:], wout[:])
    nc.sync.dma_start(iq[:], iout[:])


def shard_inputs(hidden_states, gate_weight):
    hs = np.ascontiguousarray(
        np.asarray(hidden_states, dtype=np.float32)
    ).reshape(TOKENS_TOTAL, H)
    gw = np.ascontiguousarray(np.asarray(gate_weight, dtype=np.float32))
    return [
        {"x": hs[c * TOKENS_PER_CORE : (c + 1) * TOKENS_PER_CORE], "w": gw}
        for c in range(N_CORES)
    ]


def assemble(results):
    ws, idxs = [], []
    for c in range(N_CORES):
        wqv = np.asarray(results[c]["wq"]).reshape(P, N_TILES, 2)
        iqv = np.asarray(results[c]["iq"]).reshape(P, N_TILES, 2)
        # token (core-local) = tile*128 + p
        ws.append(np.transpose(wqv, (1, 0, 2)).reshape(TOKENS_PER_CORE, 2))
        idxs.append(np.transpose(iqv, (1, 0, 2)).reshape(TOKENS_PER_CORE, 2))
    w_full = np.concatenate(ws, 0).reshape(4, 4096, 2).astype(np.float32)
    i_full = np.concatenate(idxs, 0).reshape(4, 4096, 2).astype(np.int32)
    return w_full, i_full


def kernel(hidden_states, gate_weight):
    from concourse.bass_utils import run_bass_kernel_spmd

    nc = build_program(**BEST_CONFIG)
    in_maps = shard_inputs(hidden_states, gate_weight)
    br = run_bass_kernel_spmd(nc, in_maps, list(range(N_CORES)), trace=False)
    return assemble(br.results)
